# revision 20
# baseline (speedup 1.0000x reference)
"""Trainium2 Bass kernel for nn_AccumulatorCell (histogram_binning).

Math: reference output O[b, i*180+j] = sum_t w[b,t] * e0[(p_t-i)%180] * e1[(q_t-i-j)%180]
  where w = signal_ch0 * valid, p_t/q_t = (loc-1)%180 (loc values are integers in [0,180]),
  e[d] = exp(-a * (min(d,180-d)/90)^2).

Factorization (exact):
  H[b,p,q]   = sum_t w[b,t] [p_t=p][q_t=q]          (per-batch 180x180 weighted histogram)
  S_T[b,q,i] = sum_p H[b,p,q] * G0[p,i]             (G0[p,i] = e0[(p-i)%180], circulant)
  Op[b,i,m]  = sum_q S_T[b,q,i] * G1[q,m]           (G1[q,m] = e1[(q-m)%180], circulant)
  O[b,i,j]   = Op[b,i,(i+j)%180]                    (fixed output permutation)

Device (8 cores, data parallel over batch: 16 batches/core): two bf16 matmul
stages on the PE (fp32 PSUM accumulate). All matmuls use K=128 contraction:
the 180-long contraction is zero-padded to 256 host-side (H and G uploaded
with 256 rows), so the PE never reconfigures K. A dummy-matmul burst during
the input DMA wait warms the PE clock (HAM). The final fixed permutation is
applied while unsharding.
"""

import sys

import numpy as np

for _p in ("/opt/trn_rl_repo",):
    if _p not in sys.path:
        sys.path.insert(0, _p)

import concourse.bacc as bacc
import concourse.mybir as mybir
from concourse.tile import TileContext
from concourse.bass_utils import run_bass_kernel_spmd

F32 = mybir.dt.float32
BF16 = mybir.dt.bfloat16

N_CORES = 8
B, T, CH = 128, 512, 6
LOCS, HALF, U = 180, 90, 180
U2 = U * U
BPC = B // N_CORES  # 16 batches per core
PP = 256  # contraction dim padded (2 x K=128)

_cache = {}


def _build_nc():
    nc = bacc.Bacc()
    # host pre-arranges h/g into the exact SBUF tile layouts (2D DMAs)
    h = nc.dram_tensor("h", [8, 128, 2 * (BPC // 8) * U], BF16, kind="ExternalInput")
    g = nc.dram_tensor("g", [128, 4 * U], BF16, kind="ExternalInput")
    o = nc.dram_tensor("o", [BPC, U, U], F32, kind="ExternalOutput")

    MC = [(0, 128), (128, 52)]  # output-partition chunks of the 180 dim
    GRP = 2        # batches per PSUM bank (windows at 0 and 180 within 512)
    HPIECES = 8    # h input split (batches per piece = BPC // HPIECES)
    OPIECES = 8    # output staging split
    BPP = BPC // HPIECES
    BPO = BPC // OPIECES

    with TileContext(nc) as tc:
        with tc.tile_pool(name="const", bufs=1) as cpool, tc.tile_pool(
            name="psum", bufs=2, space="PSUM"
        ) as psum:
            # histogram piece 0 first (gates the first real matmuls)
            h_all = []
            ht0 = cpool.tile([128, 2 * (BPC // 8) * U], BF16, tag="h_0")
            nc.sync.dma_start(out=ht0, in_=h[0, :, :])
            h_all.append(ht0)

            # g tile: [128, (side 2, chunk 2, col 180)] - one DMA
            gt = cpool.tile([128, 4 * U], BF16, tag="gt")
            nc.sync.dma_start(out=gt, in_=g[:, :])
            # slices: g0 chunks = [:, 0:U], [:, U:2U]; g1 chunks = [:, 2U:3U], [:, 3U:4U]
            g0t = [gt[:, 0:U], gt[:, U : 2 * U]]
            g1t = [gt[:, 2 * U : 3 * U], gt[:, 3 * U : 4 * U]]

            # remaining histogram pieces
            for pc in range(1, HPIECES):
                ht = cpool.tile([128, 2 * BPP * U], BF16, tag=f"h_{pc}")
                nc.sync.dma_start(out=ht, in_=h[pc, :, :])
                h_all.append(ht)

            def h_slice(b, cj, q0, qn):
                pc, bo = divmod(b, BPP)
                off = (cj * BPP + bo) * U
                return h_all[pc][:, off + q0 : off + q0 + qn]

            # stage-2 lhsT tiles (2 slots); chunk2 rows 52:128 zeroed once
            sT_zero = []
            for gslot in range(3):
                st1 = cpool.tile([128, GRP * U], BF16, tag=f"sT1_{gslot}")
                st2 = cpool.tile([128, GRP * U], BF16, tag=f"sT2_{gslot}")
                nc.gpsimd.memset(st2[:, :], 0.0)
                sT_zero.append((st1, st2))

            # output staging pieces
            o_all = [[], []]
            for pc in range(OPIECES):
                ot0 = cpool.tile([128, BPO * U], F32, tag=f"o_0_{pc}")
                o_all[0].append(ot0)
                ot1 = cpool.tile([52, BPO * U], F32, tag=f"o_1_{pc}")
                o_all[1].append(ot1)

            NG = BPC // GRP

            def emit_stage1(grp):
                bs = [grp * GRP + k for k in range(GRP)]
                ps1 = []
                for ci, (q0, qn) in enumerate(MC):
                    ps = psum.tile([qn, 512], F32, tag=f"s1_{ci}", name=f"ps1_{grp}_{ci}")
                    for k, b in enumerate(bs):
                        for cj in range(2):
                            nc.tensor.matmul(
                                ps[:, k * U : (k + 1) * U],
                                h_slice(b, cj, q0, qn),
                                g0t[cj],
                                start=(cj == 0),
                                stop=(cj == 1),
                            )
                    ps1.append(ps)
                sT = list(sT_zero[grp % 3])
                for ci, (q0, qn) in enumerate(MC):
                    dst = sT[ci][0:qn, :]
                    if (grp + ci) % 2 == 0:
                        nc.vector.tensor_copy(dst, ps1[ci][:, 0 : GRP * U])
                    else:
                        nc.scalar.activation(
                            dst, ps1[ci][:, 0 : GRP * U], mybir.ActivationFunctionType.Copy
                        )
                return sT

            def emit_stage2(grp, sT):
                bs = [grp * GRP + k for k in range(GRP)]
                ps2 = []
                for ci, (i0, inn) in enumerate(MC):
                    ps = psum.tile([inn, 512], F32, tag=f"s2_{ci}", name=f"ps2_{grp}_{ci}")
                    for k, b in enumerate(bs):
                        for cj in range(2):
                            nc.tensor.matmul(
                                ps[:, k * U : (k + 1) * U],
                                sT[cj][:, k * U + i0 : k * U + i0 + inn],
                                g1t[cj],
                                start=(cj == 0),
                                stop=(cj == 1),
                            )
                    ps2.append(ps)
                opiece, og = divmod(bs[0], BPO)
                ooff = og * U
                for ci, (i0, inn) in enumerate(MC):
                    dst = o_all[ci][opiece][:, ooff : ooff + GRP * U]
                    if (grp + ci) % 2 == 1:
                        nc.vector.tensor_copy(dst, ps2[ci][:, 0 : GRP * U])
                    else:
                        nc.scalar.activation(
                            dst, ps2[ci][:, 0 : GRP * U], mybir.ActivationFunctionType.Copy
                        )
                if (bs[-1] + 1) % BPO == 0:
                    for ci, (i0, inn) in enumerate(MC):
                        odma = nc.scalar if (opiece + ci) % 2 else nc.sync
                        odma.dma_start(
                            out=o[opiece * BPO : (opiece + 1) * BPO, i0 : i0 + inn, :]
                            .transpose([1, 0, 2]),
                            in_=o_all[ci][opiece].rearrange("p (b q) -> p b q", b=BPO),
                        )

            sT_prev = emit_stage1(0)
            for grp in range(NG):
                sT_next = emit_stage1(grp + 1) if grp + 1 < NG else None
                emit_stage2(grp, sT_prev)
                sT_prev = sT_next

    nc.compile()
    return nc


def _get_nc():
    if "nc" not in _cache:
        _cache["nc"] = _build_nc()
    return _cache["nc"]


def _prep(inputs, a0, a1):
    """Host prep: histogram per batch + circulant tables. Returns in_maps."""
    import ml_dtypes

    inp = np.ascontiguousarray(inputs, dtype=np.float32)
    sig0 = inp[:, :, 0]
    loc = inp[:, :, 4:6]
    valid = (loc[:, :, 0] > 0) & (loc[:, :, 1] > 0)
    w = np.where(valid, sig0, np.float32(0.0)).astype(np.float32)
    L = loc.astype(np.int32)
    p = (L[:, :, 0] - 1) % U
    q = (L[:, :, 1] - 1) % U
    H = np.zeros((B, PP, U), dtype=np.float32)
    np.add.at(H, (np.arange(B)[:, None], p, q), w)
    # rearrange per core into SBUF tile layout: [4 pieces, 128 p, (2 c, BPP b, U q)]
    BPP_ = BPC // 8
    Hb = H.astype(ml_dtypes.bfloat16)

    av0 = float(np.asarray(a0).reshape(-1)[0])
    av1 = float(np.asarray(a1).reshape(-1)[0])
    d = np.arange(U, dtype=np.float64)
    tri = np.minimum(d, U - d) / HALF
    e0 = np.exp(-av0 * tri**2)
    e1 = np.exp(-av1 * tri**2)
    idx = (np.arange(U)[:, None] - np.arange(U)[None, :]) % U
    G = np.zeros((2, PP, U), dtype=ml_dtypes.bfloat16)
    G[0, :U, :] = e0[idx].astype(ml_dtypes.bfloat16)
    G[1, :U, :] = e1[idx].astype(ml_dtypes.bfloat16)

    Gt = np.ascontiguousarray(
        G.reshape(2, 2, 128, U).transpose(2, 0, 1, 3).reshape(128, 4 * U)
    )
    in_maps = []
    for c in range(N_CORES):
        hc = Hb[c * BPC : (c + 1) * BPC]  # [BPC, 256, 180]
        ht = np.ascontiguousarray(
            hc.reshape(8, BPP_, 2, 128, U).transpose(0, 3, 2, 1, 4).reshape(8, 128, 2 * BPP_ * U)
        )
        in_maps.append({"h": ht, "g": Gt})
    return in_maps


_ROLL = ((np.arange(U)[:, None] + np.arange(U)[None, :]) % U).astype(np.int32)


def _unshard(results):
    out = np.empty((B, U2), dtype=np.float32)
    ii = np.arange(U)[:, None]
    for c, res in enumerate(results):
        op = res["o"]  # [BPC, 180, 180]
        rolled = op[:, ii, _ROLL]  # O[b,i,j] = Op[b,i,(i+j)%180]
        out[c * BPC : (c + 1) * BPC] = rolled.reshape(BPC, U2)
    return out


def run(inputs, a0, a1, **run_kwargs):
    nc = _get_nc()
    in_maps = _prep(inputs, a0, a1)
    r = run_bass_kernel_spmd(nc, in_maps, core_ids=list(range(N_CORES)), **run_kwargs)
    return _unshard(r.results), r


def kernel(inputs, a0, a1):
    out, _ = run(inputs, a0, a1)
    return out


if __name__ == "__main__":
    rng = np.random.default_rng(1)
    x = rng.standard_normal((B, T, CH)).astype(np.float32)
    x[:, :, 4:6] = rng.integers(0, LOCS + 1, size=(B, T, 2)).astype(np.float32)
    a = np.full((1,), 10.0, np.float32)
    out = kernel(x, a, a)
    print("ran:", out.shape, out.dtype)


# revision 21
# speedup vs baseline: 1.0177x; 1.0177x over previous
"""Trainium2 Bass kernel for nn_AccumulatorCell (histogram_binning).

Math: reference output O[b, i*180+j] = sum_t w[b,t] * e0[(p_t-i)%180] * e1[(q_t-i-j)%180]
  where w = signal_ch0 * valid, p_t/q_t = (loc-1)%180 (loc values are integers in [0,180]),
  e[d] = exp(-a * (min(d,180-d)/90)^2).

Factorization (exact):
  H[b,p,q]   = sum_t w[b,t] [p_t=p][q_t=q]          (per-batch 180x180 weighted histogram)
  S_T[b,q,i] = sum_p H[b,p,q] * G0[p,i]             (G0[p,i] = e0[(p-i)%180], circulant)
  Op[b,i,m]  = sum_q S_T[b,q,i] * G1[q,m]           (G1[q,m] = e1[(q-m)%180], circulant)
  O[b,i,j]   = Op[b,i,(i+j)%180]                    (fixed output permutation)

Device (8 cores, data parallel over batch: 16 batches/core): two bf16 matmul
stages on the PE (fp32 PSUM accumulate). All matmuls use K=128 contraction:
the 180-long contraction is zero-padded to 256 host-side (H and G uploaded
with 256 rows), so the PE never reconfigures K. A dummy-matmul burst during
the input DMA wait warms the PE clock (HAM). The final fixed permutation is
applied while unsharding.
"""

import sys

import numpy as np

for _p in ("/opt/trn_rl_repo",):
    if _p not in sys.path:
        sys.path.insert(0, _p)

import concourse.bacc as bacc
import concourse.mybir as mybir
from concourse.tile import TileContext
from concourse.bass_utils import run_bass_kernel_spmd

F32 = mybir.dt.float32
BF16 = mybir.dt.bfloat16

N_CORES = 8
B, T, CH = 128, 512, 6
LOCS, HALF, U = 180, 90, 180
U2 = U * U
BPC = B // N_CORES  # 16 batches per core
PP = 256  # contraction dim padded (2 x K=128)

_cache = {}


def _build_nc():
    nc = bacc.Bacc()
    # host pre-arranges h/g into the exact SBUF tile layouts (2D DMAs)
    h = nc.dram_tensor("h", [8, 128, 2 * (BPC // 8) * U], BF16, kind="ExternalInput")
    g = nc.dram_tensor("g", [128, 4 * U], BF16, kind="ExternalInput")
    o = nc.dram_tensor("o", [BPC, U, U], F32, kind="ExternalOutput")

    MC = [(0, 128), (128, 52)]  # output-partition chunks of the 180 dim
    GRP = 2        # batches per PSUM bank (windows at 0 and 180 within 512)
    HPIECES = 8    # h input split (batches per piece = BPC // HPIECES)
    OPIECES = 8    # output staging split
    BPP = BPC // HPIECES
    BPO = BPC // OPIECES

    with TileContext(nc) as tc:
        with tc.tile_pool(name="const", bufs=1) as cpool, tc.tile_pool(
            name="psum", bufs=2, space="PSUM"
        ) as psum:
            # PE warmup on a DMA-independent tile (memset-born garbage-free)
            wtile = cpool.tile([128, 640], BF16, tag="wtile")
            nc.gpsimd.memset(wtile[:, :], 0.0)
            wps = psum.tile([128, 512], F32, tag="s2_1")
            for r in range(6):
                nc.tensor.matmul(
                    wps, wtile[:, 0:128], wtile[:, 128:640], start=(r == 0), stop=(r == 5)
                )

            # histogram piece 0 first (gates the first real matmuls)
            h_all = []
            ht0 = cpool.tile([128, 2 * (BPC // 8) * U], BF16, tag="h_0")
            nc.sync.dma_start(out=ht0, in_=h[0, :, :])
            h_all.append(ht0)

            # g tile: [128, (side 2, chunk 2, col 180)] - one DMA
            gt = cpool.tile([128, 4 * U], BF16, tag="gt")
            nc.sync.dma_start(out=gt, in_=g[:, :])
            # slices: g0 chunks = [:, 0:U], [:, U:2U]; g1 chunks = [:, 2U:3U], [:, 3U:4U]
            g0t = [gt[:, 0:U], gt[:, U : 2 * U]]
            g1t = [gt[:, 2 * U : 3 * U], gt[:, 3 * U : 4 * U]]

            # remaining histogram pieces
            for pc in range(1, HPIECES):
                ht = cpool.tile([128, 2 * BPP * U], BF16, tag=f"h_{pc}")
                nc.sync.dma_start(out=ht, in_=h[pc, :, :])
                h_all.append(ht)

            def h_slice(b, cj, q0, qn):
                pc, bo = divmod(b, BPP)
                off = (cj * BPP + bo) * U
                return h_all[pc][:, off + q0 : off + q0 + qn]

            # stage-2 lhsT tiles (2 slots); chunk2 rows 52:128 zeroed once
            sT_zero = []
            for gslot in range(3):
                st1 = cpool.tile([128, GRP * U], BF16, tag=f"sT1_{gslot}")
                st2 = cpool.tile([128, GRP * U], BF16, tag=f"sT2_{gslot}")
                nc.gpsimd.memset(st2[:, :], 0.0)
                sT_zero.append((st1, st2))

            # output staging pieces
            o_all = [[], []]
            for pc in range(OPIECES):
                ot0 = cpool.tile([128, BPO * U], F32, tag=f"o_0_{pc}")
                o_all[0].append(ot0)
                ot1 = cpool.tile([52, BPO * U], F32, tag=f"o_1_{pc}")
                o_all[1].append(ot1)

            NG = BPC // GRP

            def emit_stage1(grp):
                bs = [grp * GRP + k for k in range(GRP)]
                ps1 = []
                for ci, (q0, qn) in enumerate(MC):
                    ps = psum.tile([qn, 512], F32, tag=f"s1_{ci}", name=f"ps1_{grp}_{ci}")
                    for k, b in enumerate(bs):
                        for cj in range(2):
                            nc.tensor.matmul(
                                ps[:, k * U : (k + 1) * U],
                                h_slice(b, cj, q0, qn),
                                g0t[cj],
                                start=(cj == 0),
                                stop=(cj == 1),
                            )
                    ps1.append(ps)
                sT = list(sT_zero[grp % 3])
                for ci, (q0, qn) in enumerate(MC):
                    dst = sT[ci][0:qn, :]
                    if (grp + ci) % 2 == 0:
                        nc.vector.tensor_copy(dst, ps1[ci][:, 0 : GRP * U])
                    else:
                        nc.scalar.activation(
                            dst, ps1[ci][:, 0 : GRP * U], mybir.ActivationFunctionType.Copy
                        )
                return sT

            def emit_stage2(grp, sT):
                bs = [grp * GRP + k for k in range(GRP)]
                ps2 = []
                for ci, (i0, inn) in enumerate(MC):
                    ps = psum.tile([inn, 512], F32, tag=f"s2_{ci}", name=f"ps2_{grp}_{ci}")
                    for k, b in enumerate(bs):
                        for cj in range(2):
                            nc.tensor.matmul(
                                ps[:, k * U : (k + 1) * U],
                                sT[cj][:, k * U + i0 : k * U + i0 + inn],
                                g1t[cj],
                                start=(cj == 0),
                                stop=(cj == 1),
                            )
                    ps2.append(ps)
                opiece, og = divmod(bs[0], BPO)
                ooff = og * U
                for ci, (i0, inn) in enumerate(MC):
                    dst = o_all[ci][opiece][:, ooff : ooff + GRP * U]
                    if (grp + ci) % 2 == 1:
                        nc.vector.tensor_copy(dst, ps2[ci][:, 0 : GRP * U])
                    else:
                        nc.scalar.activation(
                            dst, ps2[ci][:, 0 : GRP * U], mybir.ActivationFunctionType.Copy
                        )
                if (bs[-1] + 1) % BPO == 0:
                    for ci, (i0, inn) in enumerate(MC):
                        odma = nc.scalar if (opiece + ci) % 2 else nc.sync
                        odma.dma_start(
                            out=o[opiece * BPO : (opiece + 1) * BPO, i0 : i0 + inn, :]
                            .transpose([1, 0, 2]),
                            in_=o_all[ci][opiece].rearrange("p (b q) -> p b q", b=BPO),
                        )

            sT_prev = emit_stage1(0)
            for grp in range(NG):
                sT_next = emit_stage1(grp + 1) if grp + 1 < NG else None
                emit_stage2(grp, sT_prev)
                sT_prev = sT_next

    nc.compile()
    return nc


def _get_nc():
    if "nc" not in _cache:
        _cache["nc"] = _build_nc()
    return _cache["nc"]


def _prep(inputs, a0, a1):
    """Host prep: histogram per batch + circulant tables. Returns in_maps."""
    import ml_dtypes

    inp = np.ascontiguousarray(inputs, dtype=np.float32)
    sig0 = inp[:, :, 0]
    loc = inp[:, :, 4:6]
    valid = (loc[:, :, 0] > 0) & (loc[:, :, 1] > 0)
    w = np.where(valid, sig0, np.float32(0.0)).astype(np.float32)
    L = loc.astype(np.int32)
    p = (L[:, :, 0] - 1) % U
    q = (L[:, :, 1] - 1) % U
    H = np.zeros((B, PP, U), dtype=np.float32)
    np.add.at(H, (np.arange(B)[:, None], p, q), w)
    # rearrange per core into SBUF tile layout: [4 pieces, 128 p, (2 c, BPP b, U q)]
    BPP_ = BPC // 8
    Hb = H.astype(ml_dtypes.bfloat16)

    av0 = float(np.asarray(a0).reshape(-1)[0])
    av1 = float(np.asarray(a1).reshape(-1)[0])
    d = np.arange(U, dtype=np.float64)
    tri = np.minimum(d, U - d) / HALF
    e0 = np.exp(-av0 * tri**2)
    e1 = np.exp(-av1 * tri**2)
    idx = (np.arange(U)[:, None] - np.arange(U)[None, :]) % U
    G = np.zeros((2, PP, U), dtype=ml_dtypes.bfloat16)
    G[0, :U, :] = e0[idx].astype(ml_dtypes.bfloat16)
    G[1, :U, :] = e1[idx].astype(ml_dtypes.bfloat16)

    Gt = np.ascontiguousarray(
        G.reshape(2, 2, 128, U).transpose(2, 0, 1, 3).reshape(128, 4 * U)
    )
    in_maps = []
    for c in range(N_CORES):
        hc = Hb[c * BPC : (c + 1) * BPC]  # [BPC, 256, 180]
        ht = np.ascontiguousarray(
            hc.reshape(8, BPP_, 2, 128, U).transpose(0, 3, 2, 1, 4).reshape(8, 128, 2 * BPP_ * U)
        )
        in_maps.append({"h": ht, "g": Gt})
    return in_maps


_ROLL = ((np.arange(U)[:, None] + np.arange(U)[None, :]) % U).astype(np.int32)


def _unshard(results):
    out = np.empty((B, U2), dtype=np.float32)
    ii = np.arange(U)[:, None]
    for c, res in enumerate(results):
        op = res["o"]  # [BPC, 180, 180]
        rolled = op[:, ii, _ROLL]  # O[b,i,j] = Op[b,i,(i+j)%180]
        out[c * BPC : (c + 1) * BPC] = rolled.reshape(BPC, U2)
    return out


def run(inputs, a0, a1, **run_kwargs):
    nc = _get_nc()
    in_maps = _prep(inputs, a0, a1)
    r = run_bass_kernel_spmd(nc, in_maps, core_ids=list(range(N_CORES)), **run_kwargs)
    return _unshard(r.results), r


def kernel(inputs, a0, a1):
    out, _ = run(inputs, a0, a1)
    return out


if __name__ == "__main__":
    rng = np.random.default_rng(1)
    x = rng.standard_normal((B, T, CH)).astype(np.float32)
    x[:, :, 4:6] = rng.integers(0, LOCS + 1, size=(B, T, 2)).astype(np.float32)
    a = np.full((1,), 10.0, np.float32)
    out = kernel(x, a, a)
    print("ran:", out.shape, out.dtype)
